# revision 8
# baseline (speedup 1.0000x reference)
"""ALiBi attention (B=2, S=2048, HID=1024, H=16, D=64) on 8 TRN2 NeuronCores.

Sharding: core c -> batch b = c//4, head-group g = c%4 (4 heads = 256 dims).
Each core computes q/k/v projections for its head block, transposed-layout
attention, and a partial output projection; the host sums the 4 partials per
batch and folds the (linear-exact) bv/bo bias terms.

Math trick: softmax_j(qk/8 + slope*(j-i)) row-shifts to exp(qk/8 +
slope*(j-(S-1)) - SHIFT) / sum_j(...), whose additive term depends only on the
key index j. With scores computed transposed (keys on the partition axis),
that term is a per-partition bias folded into the ScalarE exp -- no row-max
pass, no partition-axis reductions. The softmax denominator comes from an
appended ones-column on V; normalization happens on the d=64 ctx rows.
"""

import math
from contextlib import ExitStack

import numpy as np
import ml_dtypes

import concourse.mybir as mybir
import concourse.tile as tile
from concourse import bacc
from concourse.bass_utils import run_bass_kernel_spmd

B, S, HID, H = 2, 2048, 1024, 16
D = 64
NH = 4            # heads per core
DH = NH * D       # 256 dims per core
P = 128
NCORES = 8
SHIFT = 12.0
BF16 = mybir.dt.bfloat16
F32 = mybir.dt.float32

KK = S // P       # 16 key tiles
NQ = 4            # q free chunks of 512
FD = 512


def _build(loop_r=1):
    nc = bacc.Bacc("TRN2", target_bir_lowering=False, debug=False)
    xT = nc.declare_dram_parameter("xT", [HID, S], BF16, isOutput=False)
    wq = nc.declare_dram_parameter("wqT", [HID, DH], BF16, isOutput=False)
    wk = nc.declare_dram_parameter("wkT", [HID, DH], BF16, isOutput=False)
    wv = nc.declare_dram_parameter("wvT", [HID, DH], BF16, isOutput=False)
    wo = nc.declare_dram_parameter("woS", [DH, HID], BF16, isOutput=False)
    bp = nc.declare_dram_parameter("bpack", [P, 68], F32, isOutput=False)
    out = nc.declare_dram_parameter("out", [S, HID], BF16, isOutput=True)

    Exp = mybir.ActivationFunctionType.Exp

    with tile.TileContext(nc) as tc, ExitStack() as ctx:
        if loop_r > 1:
            ctx.enter_context(tc.For_i(0, loop_r, 1))
        persist = ctx.enter_context(tc.tile_pool(name="persist", bufs=1))
        work = ctx.enter_context(tc.tile_pool(name="work", bufs=3))
        pmm = ctx.enter_context(tc.tile_pool(name="pmm", bufs=3, space="PSUM"))
        pacc = ctx.enter_context(tc.tile_pool(name="pacc", bufs=1, space="PSUM"))

        xT_sb = persist.tile([P, 8, S], BF16, tag="xT")
        wq_sb = persist.tile([P, 8, DH], BF16, tag="wq")
        wk_sb = persist.tile([P, 8, DH], BF16, tag="wk")
        wv_sb = persist.tile([P, 8, DH], BF16, tag="wv")
        wo_sb = persist.tile([P, 2, HID], BF16, tag="wo")
        bp_sb = persist.tile([P, 68], F32, tag="bp")
        qT_sb = persist.tile([P, 2, S], BF16, tag="qT")
        kT_sb = persist.tile([P, 2, S], BF16, tag="kT")
        v_sb = persist.tile([P, KK, NH, D + 1], BF16, tag="v")
        ctxT_sb = persist.tile([P, 2, S], BF16, tag="ctxT")
        ones_sb = persist.tile([1, D], F32, tag="ones")

        nc.sync.dma_start(xT_sb[:], xT[:, :].rearrange("(o p) s -> p o s", p=P))
        nc.sync.dma_start(wq_sb[:], wq[:, :].rearrange("(o p) d -> p o d", p=P))
        nc.sync.dma_start(wk_sb[:], wk[:, :].rearrange("(o p) d -> p o d", p=P))
        nc.sync.dma_start(wv_sb[:], wv[:, :].rearrange("(o p) d -> p o d", p=P))
        nc.sync.dma_start(wo_sb[:], wo[:, :].rearrange("(o p) d -> p o d", p=P))
        nc.sync.dma_start(bp_sb[:], bp[:, :])
        nc.vector.memset(ones_sb[:], 1.0)
        nc.vector.memset(v_sb[:, :, :, D:D + 1], 1.0)

        # ---- q/k projections -> feature-major qT/kT [dl, s]
        for w_sb, dst, add_bias in ((wq_sb, qT_sb, True), (wk_sb, kT_sb, False)):
            for m in range(2):
                for n in range(NQ):
                    ps = pmm.tile([P, FD], F32, tag="mm")
                    for c in range(8):
                        nc.tensor.matmul(
                            ps[:],
                            w_sb[:, c, m * P:(m + 1) * P],
                            xT_sb[:, c, n * FD:(n + 1) * FD],
                            start=(c == 0), stop=(c == 7),
                        )
                    if add_bias:
                        nc.vector.tensor_scalar_add(
                            dst[:, m, n * FD:(n + 1) * FD], ps[:],
                            bp_sb[:, 64 + m:65 + m])
                    else:
                        nc.vector.tensor_copy(
                            dst[:, m, n * FD:(n + 1) * FD], ps[:])

        # ---- v projection, token-major, interleaved per head with ones col
        for kk in range(KK):
            ps = pmm.tile([P, DH], F32, tag="mm")
            for c in range(8):
                nc.tensor.matmul(
                    ps[:],
                    xT_sb[:, c, kk * P:(kk + 1) * P],
                    wv_sb[:, c, :],
                    start=(c == 0), stop=(c == 7),
                )
            nc.vector.tensor_copy(
                v_sb[:, kk, :, 0:D],
                ps[:].rearrange("p (h d) -> p h d", h=NH))

        # ---- attention per head: scoresT -> exp(+alibi bias) -> PV accum
        for h in range(NH):
            po = D * (h % 2)
            mc = h // 2
            accs = [pacc.tile([D + 1, FD], F32, tag=f"acc{n}", name=f"acc{n}")
                    for n in range(NQ)]
            for kk in range(KK):
                for n in range(NQ):
                    st = pmm.tile([P, FD], F32, tag="mm")
                    nc.tensor.matmul(
                        st[:],
                        kT_sb[po:po + D, mc, kk * P:(kk + 1) * P],
                        qT_sb[po:po + D, mc, n * FD:(n + 1) * FD],
                        start=True, stop=True,
                    )
                    pt = work.tile([P, FD], BF16, tag="pt")
                    nc.scalar.activation(
                        pt[:], st[:], Exp,
                        bias=bp_sb[:, h * KK + kk:h * KK + kk + 1], scale=1.0)
                    nc.tensor.matmul(
                        accs[n][:], v_sb[:, kk, h, :], pt[:],
                        start=(kk == 0), stop=(kk == KK - 1),
                    )
            # normalize: ctxT[d, q] * (1/den[q]); den broadcast via K=1 matmul
            for n in range(NQ):
                den = work.tile([1, FD], F32, tag="den")
                nc.vector.tensor_copy(den[:], accs[n][D:D + 1, :])
                rec = work.tile([1, FD], F32, tag="rec")
                nc.vector.reciprocal_approx_fast(out=rec[:], in_=den[:])
                bc = pacc.tile([D, FD], F32, tag="bc")
                nc.tensor.matmul(bc[:], ones_sb[:], rec[:], start=True, stop=True)
                bcs = work.tile([D, FD], F32, tag="bcs")
                nc.vector.tensor_copy(bcs[:], bc[:])
                nc.vector.tensor_tensor(
                    ctxT_sb[po:po + D, mc, n * FD:(n + 1) * FD],
                    accs[n][0:D, :], bcs[:], mybir.AluOpType.mult)

        # ---- output projection (partial over this core's 256 dims)
        for m in range(KK):
            ob = work.tile([P, HID], BF16, tag="ob")
            for n2 in range(2):
                ps = pmm.tile([P, FD], F32, tag="mm")
                for c in range(2):
                    nc.tensor.matmul(
                        ps[:],
                        ctxT_sb[:, c, m * P:(m + 1) * P],
                        wo_sb[:, c, n2 * FD:(n2 + 1) * FD],
                        start=(c == 0), stop=(c == 1),
                    )
                nc.vector.tensor_copy(ob[:, n2 * FD:(n2 + 1) * FD], ps[:])
            nc.sync.dma_start(out[m * P:(m + 1) * P, :], ob[:])

    nc.compile()
    return nc


_nc_cache = None


def _in_map_for_core(c, x, Wq, bq, Wk, Wv, Wo, slopes):
    b, g = c // 4, c % 4
    hs = slice(g * DH, (g + 1) * DH)
    bf = ml_dtypes.bfloat16
    xTc = np.ascontiguousarray(x[b].T).astype(bf)
    wqT = np.ascontiguousarray(Wq[hs].T * 0.125).astype(bf)
    wkT = np.ascontiguousarray(Wk[hs].T).astype(bf)
    wvT = np.ascontiguousarray(Wv[hs].T).astype(bf)
    woS = np.ascontiguousarray(Wo[:, hs].T).astype(bf)
    bp = np.zeros((P, 68), np.float32)
    j = np.arange(P, dtype=np.float32)
    for hh in range(NH):
        sl = float(slopes[g * NH + hh])
        for kk in range(KK):
            bp[:, hh * KK + kk] = sl * (kk * P + j - (S - 1)) - SHIFT
    bqs = bq[hs].astype(np.float32) * 0.125
    bp[:, 64] = bqs[0:P]
    bp[:, 65] = bqs[P:2 * P]
    return {"xT": xTc, "wqT": wqT, "wkT": wkT, "wvT": wvT, "woS": woS,
            "bpack": bp}


def kernel(x, Wq, bq, Wk, bk, Wv, bv, Wo, bo, slopes):
    global _nc_cache
    x = np.asarray(x, np.float32)
    Wq = np.asarray(Wq, np.float32)
    Wk = np.asarray(Wk, np.float32)
    Wv = np.asarray(Wv, np.float32)
    Wo = np.asarray(Wo, np.float32)
    bq = np.asarray(bq, np.float32)
    bv = np.asarray(bv, np.float32)
    bo = np.asarray(bo, np.float32)
    slopes = np.asarray(slopes, np.float32)

    if _nc_cache is None:
        _nc_cache = _build()
    nc = _nc_cache

    in_maps = [_in_map_for_core(c, x, Wq, bq, Wk, Wv, Wo, slopes)
               for c in range(NCORES)]
    res = run_bass_kernel_spmd(nc, in_maps, core_ids=list(range(NCORES)))
    global LAST_RESULT
    LAST_RESULT = res

    # bk shifts every score in a row i by q_i . bk (constant over j) -> cancels
    # in softmax. bv/bo are linear post-attention terms, folded here exactly.
    bias_term = (bv @ Wo.T + bo)[None, :]
    full = np.zeros((B, S, HID), np.float32)
    for b in range(B):
        acc = np.zeros((S, HID), np.float32)
        for g in range(4):
            acc += np.asarray(res.results[b * 4 + g]["out"]).astype(np.float32)
        full[b] = acc + bias_term
    return full
